# revision 12
# baseline (speedup 1.0000x reference)
"""Trainium2 Bass kernel for nn_Activation_10445360463903.

Pipeline: 2x upsample (12-tap kaiser polyphase FIR) -> LeakyReLU(0.1) ->
2x downsample (12-tap FIR, stride 2).

v2 design (8 NeuronCores, time-sharded data parallel):
  - Host: flatten (8,128,32768)->(1024,32768); shard time into 8 slices of
    4096 outputs; per core build xt [4140, 1024] bf16 = x rows
    [4096*core-5 .. +4135) edge-clamped, transposed to [time, ch].
  - Device per core, chunk k = 0..34 (stride 118, chunk height 128):
      up:   TWO band matmuls per chunk (stationary wu_A / wu_B [128,128])
            emit psum windows ALREADY PACKED as [64 even-phase acts | 64 odd]
            x 1024 ch.  Window w covers outputs s in [59w, 59w+59).
      act:  Prelu evacuation psum->bf16 act tile [128,1024]
            (ScalarE for most windows, DVE 2-pass for some: balance).
      down: fixed stationary wd'' [128,59]; 4 MMs per window pair into ONE
            psum pair-tile (windows at partition 0:59 and 64:123 via
            tile_position col packing) -> [123, 1024] f32.
      evac: single DVE tensor_copy pair-tile -> bf16 [128,1024].
      store: 2KB-contiguous rows to out [4096, 1024] bf16 (s-major).
  - Host: gather, transpose to [ch, time], exact fixup of 3+3 edge cols.
"""
import os
import numpy as np
import ml_dtypes
from contextlib import ExitStack

import concourse.bass as bass
import concourse.bacc as bacc
import concourse.tile as tile
import concourse.mybir as mybir
from concourse.bass_utils import run_bass_kernel_spmd

# ---------------- problem constants (hardcoded per spec) ----------------
L = 32768
CH = 1024            # flattened batch*channels = 8*128
NCORES = 8
T = L // NCORES      # 4096 outputs per core
W = 59               # outputs per window
NWIN = 70            # windows per core (last partial: 25 valid)
NCHUNK = 35          # chunks per core; chunk k -> windows 2k, 2k+1
STRIDE = 118
XT_ROWS = STRIDE * (NCHUNK - 1) + 128    # 4140
SLOPE = 0.1
KSIZE = 12

BF16 = mybir.dt.bfloat16
F32 = mybir.dt.float32

NPAIR = NCHUNK                 # 35 window pairs
NGRP = (NPAIR + 1) // 2        # 18 output slab groups of 2 pairs

# which windows evacuate via DVE (2-pass) instead of ScalarE Prelu
DVE_EVERY = 8        # every 8th window


# ---------------- host-side helpers ----------------
def _build_bands(up_f, down_f):
    f = np.asarray(up_f, np.float64)
    g = np.asarray(down_f, np.float64)
    wu_a = np.zeros((128, 128))
    wu_b = np.zeros((128, 128))
    for c in range(64):
        for a in range(6):
            wu_a[c + a, c] = 2 * f[2 * a]
            wu_a[c + a, 64 + c] = 2 * f[2 * a + 1]
            wu_b[59 + c + a, c] = 2 * f[2 * a]
            wu_b[59 + c + a, 64 + c] = 2 * f[2 * a + 1]
    wd = np.zeros((128, W))
    for sp in range(W):
        for a in range(6):
            wd[sp + a, sp] = g[2 * a + 1]
            wd[64 + sp + a, sp] = g[2 * a]
    bf = ml_dtypes.bfloat16
    return wu_a.astype(bf), wu_b.astype(bf), wd.astype(bf)


def _shard_xt(x_flat, core):
    idx = np.clip(np.arange(XT_ROWS) + T * core - 5, 0, L - 1)
    return np.ascontiguousarray(x_flat[:, idx].T).astype(ml_dtypes.bfloat16)


def _edge_fixup(out_flat, x_flat, up_f, down_f):
    """Exact recompute of the 6 global-edge output columns (numpy, float64)."""
    f = np.asarray(up_f, np.float64)
    g = np.asarray(down_f, np.float64)

    def act_at(ms):
        vals = np.zeros((CH, len(ms)))
        for i, m in enumerate(ms):
            t, p = divmod(m, 2)
            acc = np.zeros(CH)
            for a in range(6):
                j = np.clip(t + a - 3 + p, 0, L - 1)
                acc += 2.0 * f[2 * a + p] * x_flat[:, j]
            vals[:, i] = acc
        return np.where(vals >= 0, vals, SLOPE * vals)

    act_lo = act_at(list(range(0, 13)))
    act_hi = act_at(list(range(2 * L - 13, 2 * L)))
    for s in list(range(3)) + list(range(L - 3, L)):
        acc = np.zeros(CH)
        for k in range(KSIZE):
            m = int(np.clip(2 * s - 5 + k, 0, 2 * L - 1))
            acc += g[k] * (act_lo[:, m] if s < 3 else act_hi[:, m - (2 * L - 13)])
        out_flat[:, s] = acc
    return out_flat


# ---------------- device kernel ----------------
def _build_nc():
    nc = bacc.Bacc()
    xt_d = nc.declare_dram_parameter("xt", [XT_ROWS, CH], BF16, isOutput=False)
    wua_d = nc.declare_dram_parameter("wua", [128, 128], BF16, isOutput=False)
    wub_d = nc.declare_dram_parameter("wub", [128, 128], BF16, isOutput=False)
    wd_d = nc.declare_dram_parameter("wd", [128, W], BF16, isOutput=False)
    # raw slab dump: group g holds pairs (2g, 2g+1); host decodes valid rows
    out_d = nc.declare_dram_parameter("out", [NGRP, 128, 2 * CH], BF16,
                                      isOutput=True)

    with ExitStack() as ctx:
        tc = ctx.enter_context(tile.TileContext(nc))
        wpool = ctx.enter_context(tc.tile_pool(name="w", bufs=1))
        xt_pool = ctx.enter_context(tc.tile_pool(name="xt", bufs=6))
        act_pool = ctx.enter_context(tc.tile_pool(name="act", bufs=12))
        tmp_pool = ctx.enter_context(tc.tile_pool(name="tmp", bufs=2))
        osb_pool = ctx.enter_context(tc.tile_pool(name="osb", bufs=6))
        ups_pool = ctx.enter_context(tc.tile_pool(name="ups", bufs=3, space="PSUM"))
        dps_pool = ctx.enter_context(tc.tile_pool(name="dps", bufs=2, space="PSUM"))

        wua_sb = wpool.tile([128, 128], BF16, name="wua_sb")
        wub_sb = wpool.tile([128, 128], BF16, name="wub_sb")
        wd_sb = wpool.tile([128, W], BF16, name="wd_sb")

        act_tiles = {}
        osb_tiles = {}
        w_loaded = []

        def emit_up(k):
            xt_t = xt_pool.tile([128, CH], BF16, name=f"xt_{k}", tag="xt")
            nc.sync.dma_start(xt_t[:], xt_d[STRIDE * k: STRIDE * k + 128, :])
            if not w_loaded:
                nc.sync.dma_start(wua_sb[:], wua_d[:])
                nc.sync.dma_start(wub_sb[:], wub_d[:])
                nc.sync.dma_start(wd_sb[:], wd_d[:])
                w_loaded.append(True)
            for wu_sb, w in ((wua_sb, 2 * k), (wub_sb, 2 * k + 1)):
                ups = ups_pool.tile([128, CH], F32, name=f"ups_{w}", tag="ups")
                nc.tensor.matmul(ups[:, 0:512], wu_sb[:], xt_t[:, 0:512],
                                 start=True, stop=True)
                nc.tensor.matmul(ups[:, 512:1024], wu_sb[:], xt_t[:, 512:1024],
                                 start=True, stop=True)
                a_t = act_pool.tile([128, CH], BF16, name=f"act_{w}", tag="act")
                if w % DVE_EVERY == DVE_EVERY - 1:
                    # DVE 2-pass lrelu: t = 0.1*u ; act = max(u, t)
                    t_t = tmp_pool.tile([128, CH], BF16, name=f"tmp_{w}",
                                        tag="tmp")
                    nc.vector.tensor_scalar_mul(t_t[:], ups[:], SLOPE)
                    nc.vector.tensor_tensor(a_t[:], ups[:], t_t[:],
                                            mybir.AluOpType.max)
                else:
                    nc.scalar.activation(a_t[:], ups[:],
                                         mybir.ActivationFunctionType.Prelu,
                                         alpha=SLOPE)
                act_tiles[w] = a_t

        def emit_down(kpair):
            w0, w1 = 2 * kpair, 2 * kpair + 1
            grp, half = divmod(kpair, 2)
            if half == 0:
                osb_tiles[grp] = osb_pool.tile([128, 2 * CH], BF16,
                                               name=f"osb_{grp}", tag="osb")
            osb = osb_tiles[grp]
            for h in (0, 1):
                c0 = 512 * h
                dps = dps_pool.tile([128, 512], F32, name=f"dps_{kpair}_{h}",
                                    tag="dps")
                nc.tensor.matmul(dps[0:W, :], wd_sb[:],
                                 act_tiles[w0][:, c0:c0 + 512],
                                 start=True, stop=True)
                nc.tensor.matmul(dps[64:64 + W, :], wd_sb[:],
                                 act_tiles[w1][:, c0:c0 + 512],
                                 start=True, stop=True)
                nc.vector.tensor_copy(osb[0:64 + W, CH * half + c0:
                                          CH * half + c0 + 512],
                                      dps[0:64 + W, :])
            del act_tiles[w0], act_tiles[w1]
            if half == 1 or kpair == NPAIR - 1:
                nc.gpsimd.dma_start(out_d[grp, :, :], osb[:, :])
                del osb_tiles[grp]

        for k in range(NCHUNK):
            if k >= 1:
                emit_down(k - 1)
            emit_up(k)
        emit_down(NCHUNK - 1)
    nc.finalize()
    return nc


_CACHE = {}


def _get_nc():
    if "nc" not in _CACHE:
        _CACHE["nc"] = _build_nc()
    return _CACHE["nc"]


# ---------------- public entry ----------------
def kernel(x, up_filter, down_filter):
    x = np.asarray(x)
    up_f = np.asarray(up_filter, np.float32)
    down_f = np.asarray(down_filter, np.float32)
    x_flat = np.ascontiguousarray(x.reshape(CH, L))

    wua, wub, wd = _build_bands(up_f, down_f)
    in_maps = []
    for core in range(NCORES):
        in_maps.append({
            "xt": _shard_xt(x_flat, core),
            "wua": wua,
            "wub": wub,
            "wd": wd,
        })

    nc = _get_nc()
    res = run_bass_kernel_spmd(nc, in_maps, core_ids=list(range(NCORES)),
                               trace=bool(os.environ.get("BASS_TRACE")))
    _CACHE["last_results"] = res
    # decode slab layout: slab[g, P, b*CH + c] -> out[s, c] with
    # s = 236g + 118b + rr, P = rr if rr < 59 else rr + 5
    s_idx = np.arange(T)
    g_i = s_idx // 236
    r_i = s_idx % 236
    b_i = r_i // 118
    rr_i = r_i % 118
    p_i = np.where(rr_i < W, rr_i, rr_i + 5)
    out_flat = np.empty((CH, L), np.float64)
    for i in range(NCORES):
        slab = np.asarray(res.results[i]["out"]).astype(np.float64)
        v = slab.reshape(NGRP, 128, 2, CH)
        out_flat[:, T * i: T * (i + 1)] = v[g_i, p_i, b_i, :].T
    out_flat = _edge_fixup(out_flat, x_flat.astype(np.float64), up_f, down_f)
    return out_flat.reshape(x.shape).astype(np.float32)


if __name__ == "__main__":
    nc = _build_nc()
    print("built ok")
